# revision 2
# baseline (speedup 1.0000x reference)
"""Circular shift kernel for Trainium2 (Bass), SPMD over 8 NeuronCores.

Reference semantics: out = vec @ roll(eye(d), -1, axis=0), which is exactly
out[b, j] = vec[b, (j-1) mod d]  (a roll by +1 along the last axis).

Sharding: data-parallel along the batch axis — each of the 8 cores handles a
contiguous [1024, 4096] row block and performs the column roll locally with
two DRAM->DRAM DMA copies:
  bulk:  out[:, 1:] = vec[:, :-1]   (row-strided, 16380 B contiguous per row)
  wrap:  out[:, 0]  = vec[:, 4095]  (1024 x 4 B strided)
The wrap DMA is serialized after the bulk one: sub-512 B HBM writes are
read-modify-write on the surrounding granule, so they must not run
concurrently with bulk writes to adjacent bytes of the same rows.
"""

import numpy as np

N_CORES = 8
ROWS = 8192
COLS = 4096
SHARD_ROWS = ROWS // N_CORES  # 1024


def _build_nc():
    import concourse.bass as bass
    import concourse.mybir as mybir

    nc = bass.Bass("TRN2")
    x = nc.dram_tensor(
        "vec", [SHARD_ROWS, COLS], mybir.dt.float32, kind="ExternalInput"
    )
    y = nc.dram_tensor(
        "out", [SHARD_ROWS, COLS], mybir.dt.float32, kind="ExternalOutput"
    )

    with nc.semaphore("dma_done") as sem:
        nc.sync.dma_start(out=y[:, 1:COLS], in_=x[:, 0 : COLS - 1]).then_inc(sem, 16)
        nc.sync.wait_ge(sem, 16)
        with nc.allow_non_contiguous_dma(reason="wrap column: 1 elem per row"):
            nc.sync.dma_start(out=y[:, 0:1], in_=x[:, COLS - 1 : COLS]).then_inc(
                sem, 16
            )
        nc.sync.wait_ge(sem, 32)
    return nc


def run(vec: np.ndarray, **spmd_kwargs):
    """Build + run the SPMD kernel; returns (full_output, BassKernelResults)."""
    from concourse import bass_utils

    vec = np.ascontiguousarray(vec, dtype=np.float32)
    assert vec.shape == (ROWS, COLS), vec.shape
    nc = _build_nc()
    in_maps = [
        {"vec": vec[i * SHARD_ROWS : (i + 1) * SHARD_ROWS]} for i in range(N_CORES)
    ]
    res = bass_utils.run_bass_kernel_spmd(
        nc, in_maps, core_ids=list(range(N_CORES)), **spmd_kwargs
    )
    out = np.concatenate([r["out"] for r in res.results], axis=0)
    return out, res


def kernel(vec: np.ndarray) -> np.ndarray:
    out, _ = run(vec)
    return out
